# revision 27
# baseline (speedup 1.0000x reference)
"""Multi-head self-attention (N=2, S=2048, E=1024, 16 heads) on 8 trn2 cores.

Sharding: data parallel over batch (2) x tensor parallel over heads (4 groups
of 4 heads). Each core computes in_proj for its local heads, attention with
full SxS scores for its local heads, and a partial out_proj (contraction over
its local 256 features). Host sums the 4 partials per batch and adds b_o.

Per-core kernel (v4, software-pipelined, ACT-bound):
  - in_proj: x/w in bf16, psum f32; q/k biased into f32r qT/kT tiles
    (features on partitions); V in natural [tok, head, dim] layout in fp8e4
    with a memset ones column per head (softmax denominators via matmul).
  - attention phases iterate head-pair outer, query-block inner; per 128-key
    tile: scores sT[k, q] = K Q^T (f32r, 512-wide moving dim), exp on ACT
    into fp8e4 ex tiles grouped in key-pair tiles [128, 2, 2hh, W].
  - attnV transposed with fp8 DoubleRow: per key-PAIR one matmul per
    (hh, 128q): stationary ex [128, 2, 128q], moving V||ones [128, 2, 65],
    256-deep contraction at 0.5 cycles/row into psum [128q, 65].
  - divide by denominator: batched per-partition reciprocal +
    tensor_scalar_mul into bf16 oT tiles [q, hd].
  - out_proj fused per 128-query block once all 4 heads are done: PE
    transpose of oT (bf16 identity), then 512-wide f32r matmuls against
    woT; partials DMA'd out per 512-col half.
  - the last query block runs as two 256-query sub-phases so its epilogue
    (divide/transpose/out_proj/DMA) is half as deep.
  Emission interleaves a deferred-work queue (attnV lagging scores, in_proj
  fillers on a cycle schedule matched to DMA arrival, transpose/out_proj
  steps gated off phase starts) to keep ACT continuously busy; dummy
  matmuls warm the PE p-state ramp while the first DMAs land.
"""
import collections
import os

import numpy as np

import concourse.bacc as bacc
import concourse.mybir as mybir
from concourse.tile import TileContext
from concourse.bass import ts

F32 = mybir.dt.float32
F32R = mybir.dt.float32r
BF16 = mybir.dt.bfloat16
F8 = mybir.dt.float8e4
EXP = mybir.ActivationFunctionType.Exp
DROW = mybir.MatmulPerfMode.DoubleRow

D_MODEL = 1024
NHEAD = 16
DH = 64
N_BATCH = 2
SEQ = 2048
N_CORES = 8
GROUPS = 4            # head groups (cores per batch)
HL = NHEAD // GROUPS  # local heads per core = 4
FL = HL * DH          # local feature width = 256

N_DUMMY = 10          # PE warm-up matmuls while the first DMAs land
DBG_STAGE = int(os.environ.get("KDBG", "9"))
DBG_POQ = int(os.environ.get("KPOQ", "99"))  # 1=inproj 2=+scores/exp 3=+attnv 4=+div 5=+tp 9=full


def build_mha(nc, S=SEQ, E=D_MODEL, EOUT=D_MODEL, scale=0.125):
    FLOC = FL                 # local q/k/v feature count (256)
    EC = E // 128             # contraction chunks for in_proj (8)
    TT = S // 128             # token tiles (16)
    KT = S // 128             # 128-wide key tiles (16)

    xT = nc.dram_tensor("xT", [E, S], BF16, kind="ExternalInput")
    wT = nc.dram_tensor("wT", [E, 3 * FLOC], BF16, kind="ExternalInput")
    qkb = nc.dram_tensor("qkb", [128, 4], F32, kind="ExternalInput")
    vbr = nc.dram_tensor("vbr", [128, HL, DH], BF16, kind="ExternalInput")
    woT = nc.dram_tensor("woT", [FLOC, EOUT], BF16, kind="ExternalInput")
    ident = nc.dram_tensor("ident", [128, 128], BF16, kind="ExternalInput")
    out = nc.dram_tensor("out", [S, EOUT], F32, kind="ExternalOutput")

    with TileContext(nc) as tc:
        with tc.tile_pool(name="pp", bufs=1) as pp, \
             tc.tile_pool(name="pw", bufs=1) as pw, \
             tc.tile_pool(name="psS", bufs=2, space="PSUM") as psS, \
             tc.tile_pool(name="psO", bufs=1, space="PSUM") as psO, \
             tc.tile_pool(name="psM", bufs=2, space="PSUM") as psM:
            xT_sb = pp.tile([128, EC, S], BF16)
            wT_sb = pp.tile([128, EC, 3 * FLOC], BF16)
            qT = pp.tile([128, 2, S], F32R)
            kT = pp.tile([128, 2, S], F32R)
            v = pp.tile([128, TT, HL, 65], BF16)
            woT_sb = pp.tile([128, 2, EOUT], BF16)
            qkb_sb = pp.tile([128, 4], F32)
            vbr_sb = pp.tile([128, HL, DH], BF16)
            ident_sb = pp.tile([128, 128], BF16)
            dums = pp.tile([128, 512], BF16)

            # ---- DMA issue order = data priority (x in 256-token chunks) --
            xTr = xT.rearrange("(c p) s -> p c s", p=128)
            wTr = wT.rearrange("(c p) f -> p c f", p=128)
            nc.sync.dma_start(xT_sb[:, :, 0:256], xTr[:, :, 0:256])
            nc.sync.dma_start(wT_sb[:, :, 256:384], wTr[:, :, 256:384])  # k ch0
            nc.sync.dma_start(xT_sb[:, :, 256:512], xTr[:, :, 256:512])
            nc.sync.dma_start(wT_sb[:, :, 0:128], wTr[:, :, 0:128])      # q ch0
            nc.sync.dma_start(qkb_sb[:], qkb[:])
            for t0 in (512, 768, 1024, 1280):
                nc.sync.dma_start(xT_sb[:, :, t0:t0 + 256], xTr[:, :, t0:t0 + 256])
            nc.sync.dma_start(wT_sb[:, :, 512:768], wTr[:, :, 512:768])  # v
            for t0 in (1536, 1792):
                nc.sync.dma_start(xT_sb[:, :, t0:t0 + 256], xTr[:, :, t0:t0 + 256])
            nc.sync.dma_start(vbr_sb[:], vbr[:])
            nc.sync.dma_start(wT_sb[:, :, 384:512], wTr[:, :, 384:512])  # k ch1
            nc.sync.dma_start(wT_sb[:, :, 128:256], wTr[:, :, 128:256])  # q ch1
            nc.sync.dma_start(ident_sb[:], ident[:])
            nc.sync.dma_start(woT_sb[:], woT.rearrange("(c p) e -> p c e", p=128))

            # ones column for the softmax denominators (cheap on DVE; a DMA
            # of 1-byte elements costs ~3.6us of descriptor time)
            nc.vector.memset(v[:, :, :, 64:65], 1.0)

            # ---- PE p-state warm-up on zeroed scratch ----
            nc.vector.memset(dums[:], 0.0)
            for _ in range(N_DUMMY):
                pm = psM.tile([128, 512], F32, tag="m", name="pdum")
                nc.tensor.matmul(pm[:], dums[:, 0:128], dums[:],
                                 start=True, stop=True)

            # ---- in_proj work units ----
            cyc = [0]        # current global kt-cycle (mutable for closures)
            v_done = [None] * TT

            def qk_tile(dst, ft, wcol, bi, t0):
                def emit():
                    pm = psM.tile([128, 512], F32, tag="m", name="pqk")
                    for c in range(EC):
                        nc.tensor.matmul(pm[:, 0:256], wT_sb[:, c, wcol:wcol + 128],
                                         xT_sb[:, c, t0:t0 + 256],
                                         start=(c == 0), stop=(c == EC - 1))
                    nc.vector.tensor_scalar_add(dst[:, ft, t0:t0 + 256],
                                                pm[:, 0:256],
                                                qkb_sb[:, bi:bi + 1])
                return emit

            def v_tile(t):
                def emit():
                    pm = psM.tile([128, 512], F32, tag="m", name="pv")
                    for c in range(EC):
                        nc.tensor.matmul(pm[:, 0:256], xT_sb[:, c, ts(t, 128)],
                                         wT_sb[:, c, 2 * FLOC:3 * FLOC],
                                         start=(c == 0), stop=(c == EC - 1))
                    nc.vector.tensor_add(
                        v[:, t, :, 0:64],
                        pm[:, 0:256].rearrange("p (h d) -> p h d", h=HL),
                        vbr_sb[:])
                    v_done[t] = cyc[0]
                return emit

            k0 = lambda t0: qk_tile(kT, 0, 256, 2, t0)
            k1 = lambda t0: qk_tile(kT, 1, 384, 3, t0)
            q0 = lambda t0: qk_tile(qT, 0, 0, 0, t0)
            q1 = lambda t0: qk_tile(qT, 1, 128, 1, t0)

            # Eager prelude: k-ch0 and q-ch0 for tokens 0:512.
            k0(0)()
            k0(256)()
            q0(0)()
            q0(256)()

            # Cycle-scheduled fillers, matched to DMA arrival and phase
            # deadlines (hp-outer: kT ch0 + q0@512:1024 by c14, v by ~c22,
            # ch1 by c62+, late q1 feeds the PE-light late phases).
            fsched = collections.deque([
                (1, k0(512)), (2, k0(768)), (3, k0(1024)), (4, k0(1280)),
                (5, q0(512)), (6, k0(1536)), (7, k0(1792)), (8, q0(768)),
                (9, v_tile(0)), (9, v_tile(1)), (10, v_tile(2)),
                (10, v_tile(3)), (11, v_tile(4)), (12, v_tile(5)),
                (13, v_tile(6)), (14, v_tile(7)), (15, v_tile(8)),
                (16, v_tile(9)), (17, v_tile(10)), (18, v_tile(11)),
                (19, v_tile(12)), (20, v_tile(13)), (21, v_tile(14)),
                (21, v_tile(15)),
                (23, q0(1024)), (25, q0(1280)),
                (28, q0(1536)), (30, q0(1792)),
                (33, q1(0)), (35, q1(256)),
                (38, k1(0)), (40, k1(256)), (42, k1(512)), (44, k1(768)),
                (47, k1(1024)), (50, k1(1280)), (53, k1(1536)), (56, k1(1792)),
                (60, q1(512)), (64, q1(768)),
                (72, q1(1024)), (80, q1(1280)),
                (88, q1(1536)), (96, q1(1792)),
            ])

            fsched = collections.deque(sorted(fsched, key=lambda x: x[0]))

            def run_fillers(maxn=2):
                n = 0
                while fsched and fsched[0][0] <= cyc[0] and n < maxn:
                    fsched.popleft()[1]()
                    n += 1

            # ---- attention pipeline state ----
            # phases: (hp, q0_off, width)
            phases = [(0, 0, 512), (0, 512, 512), (0, 1024, 512),
                      (0, 1536, 512),
                      (1, 0, 512), (1, 512, 512), (1, 1024, 512),
                      (1, 1536, 256), (1, 1792, 256)]
            NP = len(phases)
            ex_store = {}
            oacc = {}        # phase idx -> (oa, ob)
            oT_tiles = {}    # global 128-query tile index tq -> sbuf tile
            osb_tiles = {}
            pend = collections.deque()  # (kind, payload, enq_cycle)
            pstart = [0]     # cycle at which the current phase started

            def emit_attnv(P, kt):
                hp, off, W = phases[P]
                oa, ob = oacc[P]
                ex = ex_store.pop((P, kt))
                nq = W // 128
                for hh, acc in ((0, oa), (1, ob)):
                    for qq in range(nq):
                        # one accumulation group per psum BANK (2KB zero
                        # region): start only on the bank's first write,
                        # stop only on its last
                        nc.tensor.matmul(
                            acc[:, qq, :],
                            ex[:, hh, ts(qq, 128)],
                            v[:, kt, 2 * hp + hh, :],
                            start=(kt == 0 and qq == 0),
                            stop=(kt == KT - 1 and qq == nq - 1))

            def emit_recs(P, oa, ob):
                if DBG_STAGE < 4:
                    return [None, None]
                hp, off, W = phases[P]
                nq = W // 128
                recs = []
                for acc in (oa, ob):
                    rec = pw.tile([128, 4], F32, tag="rec", bufs=4,
                                  name="rec")
                    nc.vector.reciprocal(
                        rec[:, 0:nq],
                        acc[:, 0:nq, 64:65].rearrange("p q one -> p (q one)"))
                    recs.append(rec)
                return recs

            def emit_div_qq(P, qq, oa, ob, recs):
                if DBG_STAGE < 4:
                    return
                hp, off, W = phases[P]
                tq = off // 128 + qq
                if tq not in oT_tiles:
                    oT_tiles[tq] = pw.tile([128, 256], BF16, tag="ot",
                                           bufs=16, name="oT")
                oT = oT_tiles[tq]
                for hh, acc, rec in ((0, oa, recs[0]), (1, ob, recs[1])):
                    off2 = (2 * hp + hh) * 64
                    nc.vector.tensor_scalar_mul(
                        oT[:, off2:off2 + 64], acc[:, qq, 0:64],
                        rec[:, qq:qq + 1])

            def emit_div(P):
                hp, off, W = phases[P]
                oa, ob = oacc.pop(P)
                recs = emit_recs(P, oa, ob)
                for qq in range(W // 128):
                    emit_div_qq(P, qq, oa, ob, recs)

            def emit_tp(tq):
                if DBG_STAGE < 5 or tq not in oT_tiles:
                    return
                oT = oT_tiles[tq]
                osb = pw.tile([128, 2, 128], BF16, tag="osb", bufs=3,
                              name="osb")
                # one full psum bank per transpose: a second is_transpose
                # matmul at a 256B psum offset faults the exec unit
                for c in range(2):
                    tp = psM.tile([128, 128], BF16, tag="m", name="tp")
                    nc.tensor.transpose(tp[:], oT[:, ts(c, 128)], ident_sb[:])
                    nc.vector.tensor_copy(osb[:, c, :], tp[:])
                osb_tiles[tq] = osb

            def emit_po(tq):
                if DBG_STAGE < 6 or tq not in osb_tiles or tq >= DBG_POQ:
                    return
                del oT_tiles[tq]
                osb = osb_tiles.pop(tq)
                for eb in range(2):
                    pm = psM.tile([128, 512], F32, tag="m", name="po")
                    for c in range(2):
                        nc.tensor.matmul(pm[:], osb[:, c, :],
                                         woT_sb[:, c, ts(eb, 512)],
                                         start=(c == 0), stop=(c == 1))
                    fo = pw.tile([128, 512], F32, tag="fo", bufs=4, name="fo")
                    nc.vector.tensor_copy(fo[:], pm[:])
                    if DBG_STAGE >= 9:
                        nc.sync.dma_start(out[ts(tq, 128), ts(eb, 512)], fo[:])

            last_phase = [False]

            def ready(item):
                kind, payload, enq = item
                if kind == "attnv":
                    P, kt = payload
                    if kt == 0 and P > 0:
                        lag = 3
                    elif last_phase[0]:
                        lag = 1
                    else:
                        lag = 2
                    if enq > cyc[0] - lag:
                        return False
                    return v_done[kt] is not None and v_done[kt] < cyc[0]
                if kind == "div":
                    return enq < cyc[0]
                if kind == "tp" or kind == "po":
                    # keep phase starts clear for scores so ACT never starves
                    if cyc[0] - pstart[0] < 2:
                        return False
                    return enq < cyc[0] if kind == "tp" else enq <= cyc[0] - 2
                raise AssertionError(kind)

            def drain_pend(maxn):
                n = 0
                while pend and n < maxn:
                    item = pend[0]
                    if not ready(item):
                        break
                    pend.popleft()
                    kind, payload, _ = item
                    if kind == "attnv":
                        emit_attnv(*payload)
                    elif kind == "div":
                        emit_div(payload)
                    elif kind == "tp":
                        emit_tp(payload)
                    elif kind == "po":
                        emit_po(payload)
                    n += 1

            # ---- main attention loop (head-pair outer, query-block inner) --
            for P, (hp, off, W) in enumerate(phases):
                last_phase[0] = P == NP - 1
                pstart[0] = cyc[0]
                oa = psO.tile([128, 4, 65], F32, tag="oa", name="oa")
                ob = psO.tile([128, 4, 65], F32, tag="ob", name="ob")
                oacc[P] = (oa, ob)
                for kt in range(KT):
                    sps = psS.tile([128, 2, 512], F32, tag="s", name="sps")
                    for hh in range(2):
                        p0 = 64 * hh
                        nc.tensor.matmul(
                            sps[:, hh, 0:W],
                            kT[p0:p0 + 64, hp, ts(kt, 128)],
                            qT[p0:p0 + 64, hp, off:off + W],
                            start=True, stop=True)
                    ex = pw.tile([128, 2, 512], BF16, tag="ex", bufs=16,
                                 name="ex")
                    ex_store[(P, kt)] = ex
                    nc.scalar.activation(ex[:, :, 0:W], sps[:, :, 0:W],
                                         EXP, scale=scale)
                    pend.append(("attnv", (P, kt), cyc[0]))
                    cyc[0] += 1
                    drain_pend(6 if last_phase[0] else 3)
                    run_fillers()
                if P < NP - 1:
                    pend.append(("div", P, cyc[0]))
                    if hp == 1:
                        for qq in range(W // 128):
                            tq = off // 128 + qq
                            pend.append(("tp", tq, cyc[0] + qq))
                            pend.append(("po", tq, cyc[0] + qq))
            # ---- epilogue: pipelined finish of the last sub-phase ----
            while pend:
                item = pend.popleft()
                kind, payload, _ = item
                if kind == "attnv":
                    emit_attnv(*payload)
                elif kind == "div":
                    emit_div(payload)
                elif kind == "tp":
                    emit_tp(payload)
                elif kind == "po":
                    emit_po(payload)
            P = NP - 1
            hp, off, W = phases[P]
            oa, ob = oacc.pop(P)
            recs = emit_recs(P, oa, ob)
            for qq in range(W // 128):
                tq = off // 128 + qq
                emit_div_qq(P, qq, oa, ob, recs)
                if DBG_STAGE >= 5 and tq in oT_tiles:
                    emit_tp(tq)
                    emit_po(tq)
            while fsched:
                fsched.popleft()[1]()


_CACHED = {}


def _get_module():
    if "nc" not in _CACHED:
        nc = bacc.Bacc("TRN2")
        build_mha(nc)
        nc.finalize()
        _CACHED["nc"] = nc
    return _CACHED["nc"]


def make_in_maps(query, w_in, b_in, w_o):
    """Host-side sharding: per-core input dicts (layout transforms included)."""
    import ml_dtypes
    BF = ml_dtypes.bfloat16
    E, FLoc = D_MODEL, FL
    woT_full = np.ascontiguousarray(w_o.T, dtype=np.float32)  # (e_in, e_out)
    ident_arr = np.eye(128, dtype=BF)
    in_maps = []
    for core in range(N_CORES):
        b, g = divmod(core, GROUPS)
        rows = np.r_[g * FLoc:(g + 1) * FLoc,
                     E + g * FLoc:E + (g + 1) * FLoc,
                     2 * E + g * FLoc:2 * E + (g + 1) * FLoc]
        bl = b_in[rows].astype(np.float32)
        # qkb columns: q-ch0, q-ch1, k-ch0, k-ch1
        qkb_c = np.ascontiguousarray(
            np.stack([bl[0:128], bl[128:256], bl[256:384], bl[384:512]],
                     axis=1).astype(np.float32))
        vbr_c = np.ascontiguousarray(
            np.broadcast_to(bl[2 * FLoc:].reshape(1, HL, DH),
                            (128, HL, DH))).astype(BF)
        in_maps.append({
            "xT": np.ascontiguousarray(query[b].T).astype(BF),
            "wT": np.ascontiguousarray(w_in[rows].T).astype(BF),
            "qkb": qkb_c,
            "vbr": vbr_c,
            "woT": np.ascontiguousarray(woT_full[g * FLoc:(g + 1) * FLoc]).astype(BF),
            "ident": ident_arr,
        })
    return in_maps


def kernel(query, key, value, w_in, b_in, w_o, b_o, _trace=False):
    from concourse.bass_utils import run_bass_kernel_spmd
    query = np.asarray(query, dtype=np.float32)
    nc = _get_module()
    in_maps = make_in_maps(query, np.asarray(w_in), np.asarray(b_in),
                           np.asarray(w_o))
    res = run_bass_kernel_spmd(nc, in_maps, core_ids=list(range(N_CORES)),
                               trace=_trace)
    out = np.empty((N_BATCH, SEQ, D_MODEL), np.float32)
    for b in range(N_BATCH):
        acc = res.results[b * GROUPS]["out"].astype(np.float32)
        for g in range(1, GROUPS):
            acc = acc + res.results[b * GROUPS + g]["out"]
        out[b] = acc + np.asarray(b_o, dtype=np.float32)[None, :]
    if _trace:
        kernel.last_exec_time_ns = res.exec_time_ns
    return out


# revision 33
# speedup vs baseline: 1.0494x; 1.0494x over previous
"""Multi-head self-attention (N=2, S=2048, E=1024, 16 heads) on 8 trn2 cores.

Sharding: data parallel over batch (2) x tensor parallel over heads (4 groups
of 4 heads). Each core computes in_proj for its local heads, attention with
full SxS scores for its local heads, and a partial out_proj (contraction over
its local 256 features). Host sums the 4 partials per batch and adds b_o.

Per-core kernel (v4, software-pipelined, ACT-bound):
  - in_proj: x/w in bf16, psum f32; q/k biased into f32r qT/kT tiles
    (features on partitions); V in natural [tok, head, dim] layout in fp8e4
    with a memset ones column per head (softmax denominators via matmul).
  - attention phases iterate head-pair outer, query-block inner; per 128-key
    tile: scores sT[k, q] = K Q^T (f32r, 512-wide moving dim), exp on ACT
    into fp8e4 ex tiles grouped in key-pair tiles [128, 2, 2hh, W].
  - attnV transposed with fp8 DoubleRow: per key-PAIR one matmul per
    (hh, 128q): stationary ex [128, 2, 128q], moving V||ones [128, 2, 65],
    256-deep contraction at 0.5 cycles/row into psum [128q, 65].
  - divide by denominator: batched per-partition reciprocal +
    tensor_scalar_mul into bf16 oT tiles [q, hd].
  - out_proj fused per 128-query block once all 4 heads are done: PE
    transpose of oT (bf16 identity), then 512-wide f32r matmuls against
    woT; partials DMA'd out per 512-col half.
  - the last query block runs as two 256-query sub-phases so its epilogue
    (divide/transpose/out_proj/DMA) is half as deep.
  Emission interleaves a deferred-work queue (attnV lagging scores, in_proj
  fillers on a cycle schedule matched to DMA arrival, transpose/out_proj
  steps gated off phase starts) to keep ACT continuously busy; dummy
  matmuls warm the PE p-state ramp while the first DMAs land.
"""
import collections
import os

import numpy as np

import concourse.bacc as bacc
import concourse.mybir as mybir
from concourse.tile import TileContext
from concourse.bass import ts

F32 = mybir.dt.float32
F32R = mybir.dt.float32r
BF16 = mybir.dt.bfloat16
F8 = mybir.dt.float8e4
EXP = mybir.ActivationFunctionType.Exp
DROW = mybir.MatmulPerfMode.DoubleRow

D_MODEL = 1024
NHEAD = 16
DH = 64
N_BATCH = 2
SEQ = 2048
N_CORES = 8
GROUPS = 4            # head groups (cores per batch)
HL = NHEAD // GROUPS  # local heads per core = 4
FL = HL * DH          # local feature width = 256

N_DUMMY = 10          # PE warm-up matmuls while the first DMAs land
DBG_STAGE = int(os.environ.get("KDBG", "9"))
DBG_POQ = int(os.environ.get("KPOQ", "99"))  # 1=inproj 2=+scores/exp 3=+attnv 4=+div 5=+tp 9=full


def build_mha(nc, S=SEQ, E=D_MODEL, EOUT=D_MODEL, scale=0.125):
    FLOC = FL                 # local q/k/v feature count (256)
    EC = E // 128             # contraction chunks for in_proj (8)
    TT = S // 128             # token tiles (16)
    KT = S // 128             # 128-wide key tiles (16)

    xT = nc.dram_tensor("xT", [E, S], BF16, kind="ExternalInput")
    wT = nc.dram_tensor("wT", [E, 3 * FLOC], BF16, kind="ExternalInput")
    qkb = nc.dram_tensor("qkb", [128, 4], F32, kind="ExternalInput")
    vbr = nc.dram_tensor("vbr", [128, HL, DH], BF16, kind="ExternalInput")
    woT = nc.dram_tensor("woT", [FLOC, EOUT], BF16, kind="ExternalInput")
    ident = nc.dram_tensor("ident", [128, 128], BF16, kind="ExternalInput")
    out = nc.dram_tensor("out", [S, EOUT], F32, kind="ExternalOutput")

    with TileContext(nc) as tc:
        with tc.tile_pool(name="pp", bufs=1) as pp, \
             tc.tile_pool(name="pw", bufs=1) as pw, \
             tc.tile_pool(name="psS", bufs=2, space="PSUM") as psS, \
             tc.tile_pool(name="psO", bufs=1, space="PSUM") as psO, \
             tc.tile_pool(name="psM", bufs=2, space="PSUM") as psM:
            xT_sb = pp.tile([128, EC, S], BF16)
            wT_sb = pp.tile([128, EC, 3 * FLOC], BF16)
            qT = pp.tile([128, 2, S], F32R)
            kT = pp.tile([128, 2, S], F32R)
            v = pp.tile([128, TT, HL, 65], BF16)
            woT_sb = pp.tile([128, 2, EOUT], BF16)
            qkb_sb = pp.tile([128, 4], F32)
            vbr_sb = pp.tile([128, HL, DH], BF16)
            ident_sb = pp.tile([128, 128], BF16)
            dums = pp.tile([128, 512], BF16)

            # ---- DMA issue order = data priority (x in 256-token chunks) --
            xTr = xT.rearrange("(c p) s -> p c s", p=128)
            wTr = wT.rearrange("(c p) f -> p c f", p=128)
            nc.sync.dma_start(xT_sb[:, :, 0:256], xTr[:, :, 0:256])
            nc.sync.dma_start(wT_sb[:, :, 256:384], wTr[:, :, 256:384])  # k ch0
            nc.sync.dma_start(xT_sb[:, :, 256:512], xTr[:, :, 256:512])
            nc.sync.dma_start(wT_sb[:, :, 0:128], wTr[:, :, 0:128])      # q ch0
            nc.sync.dma_start(qkb_sb[:], qkb[:])
            for t0 in (512, 768, 1024, 1280):
                nc.sync.dma_start(xT_sb[:, :, t0:t0 + 256], xTr[:, :, t0:t0 + 256])
            nc.sync.dma_start(wT_sb[:, :, 512:768], wTr[:, :, 512:768])  # v
            for t0 in (1536, 1792):
                nc.sync.dma_start(xT_sb[:, :, t0:t0 + 256], xTr[:, :, t0:t0 + 256])
            nc.sync.dma_start(vbr_sb[:], vbr[:])
            nc.sync.dma_start(wT_sb[:, :, 384:512], wTr[:, :, 384:512])  # k ch1
            nc.sync.dma_start(wT_sb[:, :, 128:256], wTr[:, :, 128:256])  # q ch1
            nc.sync.dma_start(ident_sb[:], ident[:])
            nc.sync.dma_start(woT_sb[:], woT.rearrange("(c p) e -> p c e", p=128))

            # ones column for the softmax denominators (cheap on DVE; a DMA
            # of 1-byte elements costs ~3.6us of descriptor time)
            nc.vector.memset(v[:, :, :, 64:65], 1.0)

            # ---- PE p-state warm-up on zeroed scratch ----
            nc.vector.memset(dums[:], 0.0)
            for _ in range(N_DUMMY):
                pm = psM.tile([128, 512], F32, tag="m", name="pdum")
                nc.tensor.matmul(pm[:], dums[:, 0:128], dums[:],
                                 start=True, stop=True)

            # ---- in_proj work units ----
            cyc = [0]        # current global kt-cycle (mutable for closures)
            v_done = [None] * TT

            def qk_tile(dst, ft, wcol, bi, t0):
                def emit():
                    pm = psM.tile([128, 512], F32, tag="m", name="pqk")
                    for c in range(EC):
                        nc.tensor.matmul(pm[:, 0:256], wT_sb[:, c, wcol:wcol + 128],
                                         xT_sb[:, c, t0:t0 + 256],
                                         start=(c == 0), stop=(c == EC - 1))
                    nc.vector.tensor_scalar_add(dst[:, ft, t0:t0 + 256],
                                                pm[:, 0:256],
                                                qkb_sb[:, bi:bi + 1])
                return emit

            def v_tile(t):
                def emit():
                    pm = psM.tile([128, 512], F32, tag="m", name="pv")
                    for c in range(EC):
                        nc.tensor.matmul(pm[:, 0:256], xT_sb[:, c, ts(t, 128)],
                                         wT_sb[:, c, 2 * FLOC:3 * FLOC],
                                         start=(c == 0), stop=(c == EC - 1))
                    nc.vector.tensor_add(
                        v[:, t, :, 0:64],
                        pm[:, 0:256].rearrange("p (h d) -> p h d", h=HL),
                        vbr_sb[:])
                    v_done[t] = cyc[0]
                return emit

            k0 = lambda t0: qk_tile(kT, 0, 256, 2, t0)
            k1 = lambda t0: qk_tile(kT, 1, 384, 3, t0)
            q0 = lambda t0: qk_tile(qT, 0, 0, 0, t0)
            q1 = lambda t0: qk_tile(qT, 1, 128, 1, t0)

            # Eager prelude: k-ch0 and q-ch0 for tokens 0:512.
            k0(0)()
            k0(256)()
            q0(0)()
            q0(256)()

            # Cycle-scheduled fillers, matched to DMA arrival and phase
            # deadlines (hp-outer: kT ch0 + q0@512:1024 by c14, v by ~c22,
            # ch1 by c62+, late q1 feeds the PE-light late phases).
            fsched = collections.deque([
                (1, k0(512)), (2, k0(768)), (3, k0(1024)), (4, k0(1280)),
                (5, q0(512)), (6, k0(1536)), (7, k0(1792)), (8, q0(768)),
                (9, v_tile(0)), (9, v_tile(1)), (10, v_tile(2)),
                (10, v_tile(3)), (11, v_tile(4)), (12, v_tile(5)),
                (13, v_tile(6)), (14, v_tile(7)), (15, v_tile(8)),
                (16, v_tile(9)), (17, v_tile(10)), (18, v_tile(11)),
                (19, v_tile(12)), (20, v_tile(13)), (21, v_tile(14)),
                (21, v_tile(15)),
                (23, q0(1024)), (25, q0(1280)),
                (28, q0(1536)), (30, q0(1792)),
                (33, q1(0)), (35, q1(256)),
                (38, k1(0)), (40, k1(256)), (42, k1(512)), (44, k1(768)),
                (47, k1(1024)), (50, k1(1280)), (53, k1(1536)), (56, k1(1792)),
                (60, q1(512)), (64, q1(768)),
                (72, q1(1024)), (80, q1(1280)),
                (88, q1(1536)), (96, q1(1792)),
            ])

            fsched = collections.deque(sorted(fsched, key=lambda x: x[0]))

            def run_fillers(maxn=2):
                n = 0
                while fsched and fsched[0][0] <= cyc[0] and n < maxn:
                    fsched.popleft()[1]()
                    n += 1

            # ---- attention pipeline state ----
            # phases: (hp, q0_off, width)
            phases = [(0, 0, 512), (0, 512, 512), (0, 1024, 512),
                      (0, 1536, 512),
                      (1, 0, 512), (1, 512, 512), (1, 1024, 512),
                      (1, 1536, 256), (1, 1792, 256)]
            NP = len(phases)
            ex_store = {}
            oacc = {}        # phase idx -> (oa, ob)
            oT_tiles = {}    # global 128-query tile index tq -> sbuf tile
            osb_tiles = {}
            pend = collections.deque()  # (kind, payload, enq_cycle)
            pstart = [0]     # cycle at which the current phase started

            def emit_attnv(P, kt):
                hp, off, W = phases[P]
                oa, ob = oacc[P]
                ex = ex_store.pop((P, kt))
                nq = W // 128
                for hh, acc in ((0, oa), (1, ob)):
                    for qq in range(nq):
                        # one accumulation group per psum BANK (2KB zero
                        # region): start only on the bank's first write,
                        # stop only on its last
                        nc.tensor.matmul(
                            acc[:, qq, :],
                            ex[:, hh, ts(qq, 128)],
                            v[:, kt, 2 * hp + hh, :],
                            start=(kt == 0 and qq == 0),
                            stop=(kt == KT - 1 and qq == nq - 1))

            def emit_recs(P, oa, ob):
                if DBG_STAGE < 4:
                    return [None, None]
                hp, off, W = phases[P]
                nq = W // 128
                recs = []
                for acc in (oa, ob):
                    rec = pw.tile([128, 4], F32, tag="rec", bufs=4,
                                  name="rec")
                    nc.vector.reciprocal(
                        rec[:, 0:nq],
                        acc[:, 0:nq, 64:65].rearrange("p q one -> p (q one)"))
                    recs.append(rec)
                return recs

            def emit_div_qq(P, qq, oa, ob, recs):
                if DBG_STAGE < 4:
                    return
                hp, off, W = phases[P]
                tq = off // 128 + qq
                if tq not in oT_tiles:
                    oT_tiles[tq] = pw.tile([128, 256], BF16, tag="ot",
                                           bufs=16, name="oT")
                oT = oT_tiles[tq]
                for hh, acc, rec in ((0, oa, recs[0]), (1, ob, recs[1])):
                    off2 = (2 * hp + hh) * 64
                    nc.vector.tensor_scalar_mul(
                        oT[:, off2:off2 + 64], acc[:, qq, 0:64],
                        rec[:, qq:qq + 1])

            def emit_div(P):
                hp, off, W = phases[P]
                oa, ob = oacc.pop(P)
                recs = emit_recs(P, oa, ob)
                for qq in range(W // 128):
                    emit_div_qq(P, qq, oa, ob, recs)

            def emit_tp(tq):
                if DBG_STAGE < 5 or tq not in oT_tiles:
                    return
                oT = oT_tiles[tq]
                osb = pw.tile([128, 2, 128], BF16, tag="osb", bufs=3,
                              name="osb")
                # one full psum bank per transpose: a second is_transpose
                # matmul at a 256B psum offset faults the exec unit
                for c in range(2):
                    tp = psM.tile([128, 128], BF16, tag="m", name="tp")
                    nc.tensor.transpose(tp[:], oT[:, ts(c, 128)], ident_sb[:])
                    nc.vector.tensor_copy(osb[:, c, :], tp[:])
                osb_tiles[tq] = osb

            def emit_po(tq):
                if DBG_STAGE < 6 or tq not in osb_tiles or tq >= DBG_POQ:
                    return
                del oT_tiles[tq]
                osb = osb_tiles.pop(tq)
                for eb in range(2):
                    pm = psM.tile([128, 512], F32, tag="m", name="po")
                    for c in range(2):
                        nc.tensor.matmul(pm[:], osb[:, c, :],
                                         woT_sb[:, c, ts(eb, 512)],
                                         start=(c == 0), stop=(c == 1))
                    fo = pw.tile([128, 512], F32, tag="fo", bufs=4, name="fo")
                    nc.vector.tensor_copy(fo[:], pm[:])
                    if DBG_STAGE >= 9:
                        nc.sync.dma_start(out[ts(tq, 128), ts(eb, 512)], fo[:])

            last_phase = [False]

            def ready(item):
                kind, payload, enq = item
                if kind == "attnv":
                    P, kt = payload
                    if kt == 0 and P > 0:
                        lag = 3
                    elif last_phase[0]:
                        lag = 1
                    else:
                        lag = 2
                    if enq > cyc[0] - lag:
                        return False
                    return v_done[kt] is not None and v_done[kt] < cyc[0]
                if kind == "div":
                    return enq < cyc[0]
                if kind == "tp" or kind == "po":
                    # keep phase starts clear for scores so ACT never starves
                    if cyc[0] - pstart[0] < 14:
                        return False
                    return enq < cyc[0] if kind == "tp" else enq <= cyc[0] - 2
                raise AssertionError(kind)

            def drain_pend(maxn):
                n = 0
                while pend and n < maxn:
                    item = pend[0]
                    if not ready(item):
                        break
                    pend.popleft()
                    kind, payload, _ = item
                    if kind == "attnv":
                        emit_attnv(*payload)
                    elif kind == "div":
                        emit_div(payload)
                    elif kind == "tp":
                        emit_tp(payload)
                    elif kind == "po":
                        emit_po(payload)
                    n += 1

            # ---- main attention loop (head-pair outer, query-block inner) --
            for P, (hp, off, W) in enumerate(phases):
                last_phase[0] = P == NP - 1
                pstart[0] = cyc[0]
                oa = psO.tile([128, 4, 65], F32, tag="oa", name="oa")
                ob = psO.tile([128, 4, 65], F32, tag="ob", name="ob")
                oacc[P] = (oa, ob)
                for kt in range(KT):
                    sps = psS.tile([128, 2, 512], F32, tag="s", name="sps")
                    for hh in range(2):
                        p0 = 64 * hh
                        nc.tensor.matmul(
                            sps[:, hh, 0:W],
                            kT[p0:p0 + 64, hp, ts(kt, 128)],
                            qT[p0:p0 + 64, hp, off:off + W],
                            start=True, stop=True)
                    ex = pw.tile([128, 2, 512], BF16, tag="ex", bufs=16,
                                 name="ex")
                    ex_store[(P, kt)] = ex
                    nc.scalar.activation(ex[:, :, 0:W], sps[:, :, 0:W],
                                         EXP, scale=scale)
                    pend.append(("attnv", (P, kt), cyc[0]))
                    cyc[0] += 1
                    drain_pend(6 if last_phase[0] else 3)
                    run_fillers()
                if P < NP - 1:
                    pend.append(("div", P, cyc[0]))
                    if hp == 1:
                        for qq in range(W // 128):
                            tq = off // 128 + qq
                            pend.append(("tp", tq, cyc[0] + qq))
                            pend.append(("po", tq, cyc[0] + qq))
            # ---- epilogue: pipelined finish of the last sub-phase ----
            while pend:
                item = pend.popleft()
                kind, payload, _ = item
                if kind == "attnv":
                    emit_attnv(*payload)
                elif kind == "div":
                    emit_div(payload)
                elif kind == "tp":
                    emit_tp(payload)
                elif kind == "po":
                    emit_po(payload)
            P = NP - 1
            hp, off, W = phases[P]
            oa, ob = oacc.pop(P)
            recs = emit_recs(P, oa, ob)
            for qq in range(W // 128):
                tq = off // 128 + qq
                emit_div_qq(P, qq, oa, ob, recs)
                if DBG_STAGE >= 5 and tq in oT_tiles:
                    emit_tp(tq)
                    emit_po(tq)
            while fsched:
                fsched.popleft()[1]()


_CACHED = {}


def _get_module():
    if "nc" not in _CACHED:
        nc = bacc.Bacc("TRN2")
        build_mha(nc)
        nc.finalize()
        _CACHED["nc"] = nc
    return _CACHED["nc"]


def make_in_maps(query, w_in, b_in, w_o):
    """Host-side sharding: per-core input dicts (layout transforms included)."""
    import ml_dtypes
    BF = ml_dtypes.bfloat16
    E, FLoc = D_MODEL, FL
    woT_full = np.ascontiguousarray(w_o.T, dtype=np.float32)  # (e_in, e_out)
    ident_arr = np.eye(128, dtype=BF)
    in_maps = []
    for core in range(N_CORES):
        b, g = divmod(core, GROUPS)
        rows = np.r_[g * FLoc:(g + 1) * FLoc,
                     E + g * FLoc:E + (g + 1) * FLoc,
                     2 * E + g * FLoc:2 * E + (g + 1) * FLoc]
        bl = b_in[rows].astype(np.float32)
        # qkb columns: q-ch0, q-ch1, k-ch0, k-ch1
        qkb_c = np.ascontiguousarray(
            np.stack([bl[0:128], bl[128:256], bl[256:384], bl[384:512]],
                     axis=1).astype(np.float32))
        vbr_c = np.ascontiguousarray(
            np.broadcast_to(bl[2 * FLoc:].reshape(1, HL, DH),
                            (128, HL, DH))).astype(BF)
        in_maps.append({
            "xT": np.ascontiguousarray(query[b].T).astype(BF),
            "wT": np.ascontiguousarray(w_in[rows].T).astype(BF),
            "qkb": qkb_c,
            "vbr": vbr_c,
            "woT": np.ascontiguousarray(woT_full[g * FLoc:(g + 1) * FLoc]).astype(BF),
            "ident": ident_arr,
        })
    return in_maps


def kernel(query, key, value, w_in, b_in, w_o, b_o, _trace=False):
    from concourse.bass_utils import run_bass_kernel_spmd
    query = np.asarray(query, dtype=np.float32)
    nc = _get_module()
    in_maps = make_in_maps(query, np.asarray(w_in), np.asarray(b_in),
                           np.asarray(w_o))
    res = run_bass_kernel_spmd(nc, in_maps, core_ids=list(range(N_CORES)),
                               trace=_trace)
    out = np.empty((N_BATCH, SEQ, D_MODEL), np.float32)
    for b in range(N_BATCH):
        acc = res.results[b * GROUPS]["out"].astype(np.float32)
        for g in range(1, GROUPS):
            acc = acc + res.results[b * GROUPS + g]["out"]
        out[b] = acc + np.asarray(b_o, dtype=np.float32)[None, :]
    if _trace:
        kernel.last_exec_time_ns = res.exec_time_ns
    return out


# revision 35
# speedup vs baseline: 1.0544x; 1.0047x over previous
"""Multi-head self-attention (N=2, S=2048, E=1024, 16 heads) on 8 trn2 cores.

Sharding: data parallel over batch (2) x tensor parallel over heads (4 groups
of 4 heads). Each core computes in_proj for its local heads, attention with
full SxS scores for its local heads, and a partial out_proj (contraction over
its local 256 features). Host sums the 4 partials per batch and adds b_o.

Per-core kernel (v4, software-pipelined, ACT-bound):
  - in_proj: x/w in bf16, psum f32; q/k biased into f32r qT/kT tiles
    (features on partitions); V in natural [tok, head, dim] layout in fp8e4
    with a memset ones column per head (softmax denominators via matmul).
  - attention phases iterate head-pair outer, query-block inner; per 128-key
    tile: scores sT[k, q] = K Q^T (f32r, 512-wide moving dim), exp on ACT
    into fp8e4 ex tiles grouped in key-pair tiles [128, 2, 2hh, W].
  - attnV transposed with fp8 DoubleRow: per key-PAIR one matmul per
    (hh, 128q): stationary ex [128, 2, 128q], moving V||ones [128, 2, 65],
    256-deep contraction at 0.5 cycles/row into psum [128q, 65].
  - divide by denominator: batched per-partition reciprocal +
    tensor_scalar_mul into bf16 oT tiles [q, hd].
  - out_proj fused per 128-query block once all 4 heads are done: PE
    transpose of oT (bf16 identity), then 512-wide f32r matmuls against
    woT; partials DMA'd out per 512-col half.
  - the last query block runs as two 256-query sub-phases so its epilogue
    (divide/transpose/out_proj/DMA) is half as deep.
  Emission interleaves a deferred-work queue (attnV lagging scores, in_proj
  fillers on a cycle schedule matched to DMA arrival, transpose/out_proj
  steps gated off phase starts) to keep ACT continuously busy; dummy
  matmuls warm the PE p-state ramp while the first DMAs land.
"""
import collections
import os

import numpy as np

import concourse.bacc as bacc
import concourse.mybir as mybir
from concourse.tile import TileContext
from concourse.bass import ts

F32 = mybir.dt.float32
F32R = mybir.dt.float32r
BF16 = mybir.dt.bfloat16
F8 = mybir.dt.float8e4
EXP = mybir.ActivationFunctionType.Exp
DROW = mybir.MatmulPerfMode.DoubleRow

D_MODEL = 1024
NHEAD = 16
DH = 64
N_BATCH = 2
SEQ = 2048
N_CORES = 8
GROUPS = 4            # head groups (cores per batch)
HL = NHEAD // GROUPS  # local heads per core = 4
FL = HL * DH          # local feature width = 256

N_DUMMY = 10          # PE warm-up matmuls while the first DMAs land
DBG_STAGE = int(os.environ.get("KDBG", "9"))
DBG_POQ = int(os.environ.get("KPOQ", "99"))  # 1=inproj 2=+scores/exp 3=+attnv 4=+div 5=+tp 9=full


def build_mha(nc, S=SEQ, E=D_MODEL, EOUT=D_MODEL, scale=0.125):
    FLOC = FL                 # local q/k/v feature count (256)
    EC = E // 128             # contraction chunks for in_proj (8)
    TT = S // 128             # token tiles (16)
    KT = S // 128             # 128-wide key tiles (16)

    xT = nc.dram_tensor("xT", [E, S], BF16, kind="ExternalInput")
    wT = nc.dram_tensor("wT", [E, 3 * FLOC], BF16, kind="ExternalInput")
    qkb = nc.dram_tensor("qkb", [128, 4], F32, kind="ExternalInput")
    vbr = nc.dram_tensor("vbr", [128, HL, DH], BF16, kind="ExternalInput")
    woT = nc.dram_tensor("woT", [FLOC, EOUT], BF16, kind="ExternalInput")
    ident = nc.dram_tensor("ident", [128, 128], BF16, kind="ExternalInput")
    out = nc.dram_tensor("out", [S, EOUT], F32, kind="ExternalOutput")

    with TileContext(nc) as tc:
        with tc.tile_pool(name="pp", bufs=1) as pp, \
             tc.tile_pool(name="pw", bufs=1) as pw, \
             tc.tile_pool(name="psS", bufs=2, space="PSUM") as psS, \
             tc.tile_pool(name="psO", bufs=1, space="PSUM") as psO, \
             tc.tile_pool(name="psM", bufs=2, space="PSUM") as psM:
            xT_sb = pp.tile([128, EC, S], BF16)
            wT_sb = pp.tile([128, EC, 3 * FLOC], BF16)
            qT = pp.tile([128, 2, S], F32R)
            kT = pp.tile([128, 2, S], F32R)
            v = pp.tile([128, TT, HL, 65], BF16)
            woT_sb = pp.tile([128, 2, EOUT], BF16)
            qkb_sb = pp.tile([128, 4], F32)
            vbr_sb = pp.tile([128, HL, DH], BF16)
            ident_sb = pp.tile([128, 128], BF16)
            dums = pp.tile([128, 512], BF16)

            # ---- DMA issue order = data priority (x in 256-token chunks) --
            xTr = xT.rearrange("(c p) s -> p c s", p=128)
            wTr = wT.rearrange("(c p) f -> p c f", p=128)
            nc.sync.dma_start(xT_sb[:, :, 0:256], xTr[:, :, 0:256])
            nc.sync.dma_start(wT_sb[:, :, 256:384], wTr[:, :, 256:384])  # k ch0
            nc.sync.dma_start(xT_sb[:, :, 256:512], xTr[:, :, 256:512])
            nc.sync.dma_start(wT_sb[:, :, 0:128], wTr[:, :, 0:128])      # q ch0
            nc.sync.dma_start(qkb_sb[:], qkb[:])
            for t0 in (512, 768, 1024, 1280):
                nc.sync.dma_start(xT_sb[:, :, t0:t0 + 256], xTr[:, :, t0:t0 + 256])
            nc.sync.dma_start(wT_sb[:, :, 512:768], wTr[:, :, 512:768])  # v
            for t0 in (1536, 1792):
                nc.sync.dma_start(xT_sb[:, :, t0:t0 + 256], xTr[:, :, t0:t0 + 256])
            nc.sync.dma_start(vbr_sb[:], vbr[:])
            nc.sync.dma_start(wT_sb[:, :, 384:512], wTr[:, :, 384:512])  # k ch1
            nc.sync.dma_start(wT_sb[:, :, 128:256], wTr[:, :, 128:256])  # q ch1
            nc.sync.dma_start(ident_sb[:], ident[:])
            nc.sync.dma_start(woT_sb[:], woT.rearrange("(c p) e -> p c e", p=128))

            # ones column for the softmax denominators (cheap on DVE; a DMA
            # of 1-byte elements costs ~3.6us of descriptor time)
            nc.vector.memset(v[:, :, :, 64:65], 1.0)

            # ---- PE p-state warm-up on zeroed scratch ----
            nc.vector.memset(dums[:], 0.0)
            for _ in range(N_DUMMY):
                pm = psM.tile([128, 512], F32, tag="m", name="pdum")
                nc.tensor.matmul(pm[:], dums[:, 0:128], dums[:],
                                 start=True, stop=True)

            # ---- in_proj work units ----
            cyc = [0]        # current global kt-cycle (mutable for closures)
            v_done = [None] * TT

            def qk_tile(dst, ft, wcol, bi, t0):
                def emit():
                    pm = psM.tile([128, 512], F32, tag="m", name="pqk")
                    for c in range(EC):
                        nc.tensor.matmul(pm[:, 0:256], wT_sb[:, c, wcol:wcol + 128],
                                         xT_sb[:, c, t0:t0 + 256],
                                         start=(c == 0), stop=(c == EC - 1))
                    nc.vector.tensor_scalar_add(dst[:, ft, t0:t0 + 256],
                                                pm[:, 0:256],
                                                qkb_sb[:, bi:bi + 1])
                return emit

            def v_tile(t):
                def emit():
                    pm = psM.tile([128, 512], F32, tag="m", name="pv")
                    for c in range(EC):
                        nc.tensor.matmul(pm[:, 0:256], xT_sb[:, c, ts(t, 128)],
                                         wT_sb[:, c, 2 * FLOC:3 * FLOC],
                                         start=(c == 0), stop=(c == EC - 1))
                    nc.vector.tensor_add(
                        v[:, t, :, 0:64],
                        pm[:, 0:256].rearrange("p (h d) -> p h d", h=HL),
                        vbr_sb[:])
                    v_done[t] = cyc[0]
                return emit

            k0 = lambda t0: qk_tile(kT, 0, 256, 2, t0)
            k1 = lambda t0: qk_tile(kT, 1, 384, 3, t0)
            q0 = lambda t0: qk_tile(qT, 0, 0, 0, t0)
            q1 = lambda t0: qk_tile(qT, 1, 128, 1, t0)

            # Eager prelude: k-ch0 and q-ch0 for tokens 0:512.
            k0(0)()
            k0(256)()
            q0(0)()
            q0(256)()

            # Cycle-scheduled fillers, matched to DMA arrival and phase
            # deadlines (hp-outer: kT ch0 + q0@512:1024 by c14, v by ~c22,
            # ch1 by c62+, late q1 feeds the PE-light late phases).
            fsched = collections.deque([
                (1, k0(512)), (2, k0(768)), (3, k0(1024)), (4, k0(1280)),
                (5, q0(512)), (6, k0(1536)), (7, k0(1792)), (8, q0(768)),
                (9, v_tile(0)), (9, v_tile(1)), (10, v_tile(2)),
                (10, v_tile(3)), (11, v_tile(4)), (12, v_tile(5)),
                (13, v_tile(6)), (14, v_tile(7)), (15, v_tile(8)),
                (16, v_tile(9)), (17, v_tile(10)), (18, v_tile(11)),
                (19, v_tile(12)), (20, v_tile(13)), (21, v_tile(14)),
                (21, v_tile(15)),
                (23, q0(1024)), (25, q0(1280)),
                (28, q0(1536)), (30, q0(1792)),
                (33, q1(0)), (35, q1(256)),
                (38, k1(0)), (40, k1(256)), (42, k1(512)), (44, k1(768)),
                (47, k1(1024)), (50, k1(1280)), (53, k1(1536)), (56, k1(1792)),
                (60, q1(512)), (64, q1(768)),
                (72, q1(1024)), (80, q1(1280)),
                (88, q1(1536)), (96, q1(1792)),
            ])

            fsched = collections.deque(sorted(fsched, key=lambda x: x[0]))

            def run_fillers(maxn=2):
                n = 0
                while fsched and fsched[0][0] <= cyc[0] and n < maxn:
                    fsched.popleft()[1]()
                    n += 1

            # ---- attention pipeline state ----
            # phases: (hp, q0_off, width)
            phases = [(0, 0, 512), (0, 512, 512), (0, 1024, 512),
                      (0, 1536, 512),
                      (1, 0, 512), (1, 512, 512), (1, 1024, 512),
                      (1, 1536, 256), (1, 1792, 256)]
            NP = len(phases)
            ex_store = {}
            oacc = {}        # phase idx -> (oa, ob)
            oT_tiles = {}    # global 128-query tile index tq -> sbuf tile
            osb_tiles = {}
            pend = collections.deque()  # (kind, payload, enq_cycle)
            pstart = [0]     # cycle at which the current phase started

            def emit_attnv(P, kt):
                hp, off, W = phases[P]
                oa, ob = oacc[P]
                ex = ex_store.pop((P, kt))
                nq = W // 128
                for hh, acc in ((0, oa), (1, ob)):
                    for qq in range(nq):
                        # one accumulation group per psum BANK (2KB zero
                        # region): start only on the bank's first write,
                        # stop only on its last
                        nc.tensor.matmul(
                            acc[:, qq, :],
                            ex[:, hh, ts(qq, 128)],
                            v[:, kt, 2 * hp + hh, :],
                            start=(kt == 0 and qq == 0),
                            stop=(kt == KT - 1 and qq == nq - 1))

            def emit_recs(P, oa, ob):
                if DBG_STAGE < 4:
                    return [None, None]
                hp, off, W = phases[P]
                nq = W // 128
                recs = []
                for acc in (oa, ob):
                    rec = pw.tile([128, 4], F32, tag="rec", bufs=4,
                                  name="rec")
                    nc.vector.reciprocal(
                        rec[:, 0:nq],
                        acc[:, 0:nq, 64:65].rearrange("p q one -> p (q one)"))
                    recs.append(rec)
                return recs

            def emit_div_qq(P, qq, oa, ob, recs):
                if DBG_STAGE < 4:
                    return
                hp, off, W = phases[P]
                tq = off // 128 + qq
                if tq not in oT_tiles:
                    oT_tiles[tq] = pw.tile([128, 256], BF16, tag="ot",
                                           bufs=16, name="oT")
                oT = oT_tiles[tq]
                for hh, acc, rec in ((0, oa, recs[0]), (1, ob, recs[1])):
                    off2 = (2 * hp + hh) * 64
                    nc.vector.tensor_scalar_mul(
                        oT[:, off2:off2 + 64], acc[:, qq, 0:64],
                        rec[:, qq:qq + 1])

            def emit_div(P):
                hp, off, W = phases[P]
                oa, ob = oacc.pop(P)
                recs = emit_recs(P, oa, ob)
                for qq in range(W // 128):
                    emit_div_qq(P, qq, oa, ob, recs)

            def emit_tp(tq):
                if DBG_STAGE < 5 or tq not in oT_tiles:
                    return
                oT = oT_tiles[tq]
                osb = pw.tile([128, 2, 128], BF16, tag="osb", bufs=3,
                              name="osb")
                # one full psum bank per transpose: a second is_transpose
                # matmul at a 256B psum offset faults the exec unit
                for c in range(2):
                    tp = psM.tile([128, 128], BF16, tag="m", name="tp")
                    nc.tensor.transpose(tp[:], oT[:, ts(c, 128)], ident_sb[:])
                    nc.vector.tensor_copy(osb[:, c, :], tp[:])
                osb_tiles[tq] = osb

            def emit_po(tq):
                if DBG_STAGE < 6 or tq not in osb_tiles or tq >= DBG_POQ:
                    return
                del oT_tiles[tq]
                osb = osb_tiles.pop(tq)
                for eb in range(2):
                    pm = psM.tile([128, 512], F32, tag="m", name="po")
                    for c in range(2):
                        nc.tensor.matmul(pm[:], osb[:, c, :],
                                         woT_sb[:, c, ts(eb, 512)],
                                         start=(c == 0), stop=(c == 1))
                    fo = pw.tile([128, 512], F32, tag="fo", bufs=4, name="fo")
                    nc.vector.tensor_copy(fo[:], pm[:])
                    if DBG_STAGE >= 9:
                        nc.sync.dma_start(out[ts(tq, 128), ts(eb, 512)], fo[:])

            last_phase = [False]

            def ready(item):
                kind, payload, enq = item
                if kind == "attnv":
                    P, kt = payload
                    if kt == 0 and P > 0:
                        lag = 3
                    elif last_phase[0]:
                        lag = 1
                    else:
                        lag = 2
                    if enq > cyc[0] - lag:
                        return False
                    return v_done[kt] is not None and v_done[kt] < cyc[0]
                if kind == "div":
                    return enq < cyc[0]
                if kind == "tp" or kind == "po":
                    # keep phase starts clear for scores so ACT never starves
                    if cyc[0] - pstart[0] < 16:
                        return False
                    return enq < cyc[0] if kind == "tp" else enq <= cyc[0] - 2
                raise AssertionError(kind)

            def drain_pend(maxn):
                n = 0
                while pend and n < maxn:
                    item = pend[0]
                    if not ready(item):
                        break
                    pend.popleft()
                    kind, payload, _ = item
                    if kind == "attnv":
                        emit_attnv(*payload)
                    elif kind == "div":
                        emit_div(payload)
                    elif kind == "tp":
                        emit_tp(payload)
                    elif kind == "po":
                        emit_po(payload)
                    n += 1

            # ---- main attention loop (head-pair outer, query-block inner) --
            for P, (hp, off, W) in enumerate(phases):
                last_phase[0] = P == NP - 1
                pstart[0] = cyc[0]
                oa = psO.tile([128, 4, 65], F32, tag="oa", name="oa")
                ob = psO.tile([128, 4, 65], F32, tag="ob", name="ob")
                oacc[P] = (oa, ob)
                for kt in range(KT):
                    sps = psS.tile([128, 2, 512], F32, tag="s", name="sps")
                    for hh in range(2):
                        p0 = 64 * hh
                        nc.tensor.matmul(
                            sps[:, hh, 0:W],
                            kT[p0:p0 + 64, hp, ts(kt, 128)],
                            qT[p0:p0 + 64, hp, off:off + W],
                            start=True, stop=True)
                    ex = pw.tile([128, 2, 512], BF16, tag="ex", bufs=16,
                                 name="ex")
                    ex_store[(P, kt)] = ex
                    nc.scalar.activation(ex[:, :, 0:W], sps[:, :, 0:W],
                                         EXP, scale=scale)
                    pend.append(("attnv", (P, kt), cyc[0]))
                    cyc[0] += 1
                    drain_pend(6 if last_phase[0] else 3)
                    run_fillers()
                if P < NP - 1:
                    pend.append(("div", P, cyc[0]))
                    if hp == 1:
                        for qq in range(W // 128):
                            tq = off // 128 + qq
                            pend.append(("tp", tq, cyc[0] + qq))
                            pend.append(("po", tq, cyc[0] + qq))
            # ---- epilogue: pipelined finish of the last sub-phase ----
            while pend:
                item = pend.popleft()
                kind, payload, _ = item
                if kind == "attnv":
                    emit_attnv(*payload)
                elif kind == "div":
                    emit_div(payload)
                elif kind == "tp":
                    emit_tp(payload)
                elif kind == "po":
                    emit_po(payload)
            P = NP - 1
            hp, off, W = phases[P]
            oa, ob = oacc.pop(P)
            recs = emit_recs(P, oa, ob)
            for qq in range(W // 128):
                tq = off // 128 + qq
                emit_div_qq(P, qq, oa, ob, recs)
                if DBG_STAGE >= 5 and tq in oT_tiles:
                    emit_tp(tq)
                    emit_po(tq)
            while fsched:
                fsched.popleft()[1]()


_CACHED = {}


def _get_module():
    if "nc" not in _CACHED:
        nc = bacc.Bacc("TRN2")
        build_mha(nc)
        nc.finalize()
        _CACHED["nc"] = nc
    return _CACHED["nc"]


def make_in_maps(query, w_in, b_in, w_o):
    """Host-side sharding: per-core input dicts (layout transforms included)."""
    import ml_dtypes
    BF = ml_dtypes.bfloat16
    E, FLoc = D_MODEL, FL
    woT_full = np.ascontiguousarray(w_o.T, dtype=np.float32)  # (e_in, e_out)
    ident_arr = np.eye(128, dtype=BF)
    in_maps = []
    for core in range(N_CORES):
        b, g = divmod(core, GROUPS)
        rows = np.r_[g * FLoc:(g + 1) * FLoc,
                     E + g * FLoc:E + (g + 1) * FLoc,
                     2 * E + g * FLoc:2 * E + (g + 1) * FLoc]
        bl = b_in[rows].astype(np.float32)
        # qkb columns: q-ch0, q-ch1, k-ch0, k-ch1
        qkb_c = np.ascontiguousarray(
            np.stack([bl[0:128], bl[128:256], bl[256:384], bl[384:512]],
                     axis=1).astype(np.float32))
        vbr_c = np.ascontiguousarray(
            np.broadcast_to(bl[2 * FLoc:].reshape(1, HL, DH),
                            (128, HL, DH))).astype(BF)
        in_maps.append({
            "xT": np.ascontiguousarray(query[b].T).astype(BF),
            "wT": np.ascontiguousarray(w_in[rows].T).astype(BF),
            "qkb": qkb_c,
            "vbr": vbr_c,
            "woT": np.ascontiguousarray(woT_full[g * FLoc:(g + 1) * FLoc]).astype(BF),
            "ident": ident_arr,
        })
    return in_maps


def kernel(query, key, value, w_in, b_in, w_o, b_o, _trace=False):
    from concourse.bass_utils import run_bass_kernel_spmd
    query = np.asarray(query, dtype=np.float32)
    nc = _get_module()
    in_maps = make_in_maps(query, np.asarray(w_in), np.asarray(b_in),
                           np.asarray(w_o))
    res = run_bass_kernel_spmd(nc, in_maps, core_ids=list(range(N_CORES)),
                               trace=_trace)
    out = np.empty((N_BATCH, SEQ, D_MODEL), np.float32)
    for b in range(N_BATCH):
        acc = res.results[b * GROUPS]["out"].astype(np.float32)
        for g in range(1, GROUPS):
            acc = acc + res.results[b * GROUPS + g]["out"]
        out[b] = acc + np.asarray(b_o, dtype=np.float32)[None, :]
    if _trace:
        kernel.last_exec_time_ns = res.exec_time_ns
    return out
